# revision 10
# baseline (speedup 1.0000x reference)
"""Trainium2 Bass kernel for the social-GAN LSTM decoder.

Full-input contract: kernel(**inputs) takes the unsharded numpy inputs and
returns (pred_traj_fake_rel (T,B,2), h_final (1,B,H)) exactly like the
reference. Internally: pure data-parallel over the batch across 8 cores.

Math notes (exact algebra, fp32):
  reference per step:
    gates = x @ w_ih.T + h @ w_hh.T + (b_ih + b_hh)
    i,f,g,o = split(gates); c' = sig(f)*c + sig(i)*tanh(g); h' = sig(o)*tanh(c')
    rel = h' @ w_pos.T + b_pos
    x' = rel @ w_emb.T + b_emb
  The (last_pos -> cur) chain never feeds any output, so it is dropped.
  x_{t+1} is linear in h_t, so for t>=1:
    gates_t = h_t @ W_eff.T + b_eff
      W_eff = w_hh + w_ih @ (w_emb @ w_pos)
      b_eff = b_ih + b_hh + w_ih @ (b_emb + w_emb @ b_pos)
  and for t=0 (x0 from last_pos_rel):
    gates_0 = h0 @ w_hh.T + last_pos_rel @ W0.T + b_eff,  W0 = w_ih @ w_emb

Device layout: feature-major. h,c live in SBUF as (H=128 partitions, batch
free). Gates for a batch-tile of 512 are one (128, 2048) PSUM tile in gate
order [i,f,o,g]: one fused Sigmoid over the first 1536 columns, one Tanh on
the last 512. Biases are added with K=1 matmuls (PE has slack; ACT is the
bottleneck engine). rel_t = w_pos @ h_t is an M=2 matmul written into the
already-consumed i-gate PSUM region, then DMA'd straight to DRAM.
"""

import os
import sys

import numpy as np

for _p in ("/opt/trn_rl_repo", "/root/.axon_site/_ro/trn_rl_repo"):
    if os.path.isdir(_p) and _p not in sys.path:
        try:
            import concourse.bass  # noqa: F401

            break
        except Exception:
            sys.path.insert(0, _p)

B, T, EMB, H = 65536, 32, 64, 128
NCORES = 8
BC = B // NCORES  # batch per core
BT = 512  # batch tile (free dim per matmul / PSUM bank)
G = 4 * H  # 512 gate dims, order [i, f, o, g]

LAST_RESULT = None  # BassKernelResults of the most recent run (for test.py)
LAST_NC = None  # built Bass program of the most recent run (for benching)


def _build_program(weffT, whhT, w0T, wposT, biasT, bc=BC, t_steps=T):
    """Build the per-core Bass/Tile program. Weights are baked into the NEFF
    as Const tensors; per-core tensors hh0/ch0/lpr are ExternalInputs."""
    import concourse.mybir as mybir
    from concourse import bacc
    from concourse.masks import make_identity
    from concourse.tile import TileContext

    f32 = mybir.dt.float32
    AF = mybir.ActivationFunctionType
    nbt = bc // BT

    nc = bacc.Bacc("TRN2", target_bir_lowering=False, debug=False)

    hh_d = nc.dram_tensor("hh0", [bc, H], f32, kind="ExternalInput")
    ch_d = nc.dram_tensor("ch0", [bc, H], f32, kind="ExternalInput")
    lpr_d = nc.dram_tensor("lpr", [bc, 2], f32, kind="ExternalInput")
    rels_d = nc.dram_tensor("rels", [t_steps, bc, 2], f32, kind="ExternalOutput")
    hN_d = nc.dram_tensor("hN", [bc, H], f32, kind="ExternalOutput")

    weff_dr = nc.inline_tensor(weffT, "weffT")  # (H, G)
    whh_dr = nc.inline_tensor(whhT, "whhT")  # (H, G)
    w0_dr = nc.inline_tensor(w0T, "w0T")  # (2, G)
    wpos_dr = nc.inline_tensor(wposT, "wposT")  # (H, 2)
    bias_dr = nc.inline_tensor(biasT, "biasT")  # (1, G)

    with TileContext(nc) as tc:
        with tc.tile_pool(name="persist", bufs=1) as pp:
            weff_sb = pp.tile_from(weff_dr[:, :])
            whh_sb = pp.tile_from(whh_dr[:, :])
            w0_sb = pp.tile_from(w0_dr[:, :])
            wpos_sb = pp.tile_from(wpos_dr[:, :])
            bias_sb = pp.tile_from(bias_dr[:, :])
            ident = pp.tile([128, 128], f32)
            make_identity(nc, ident)
            ones_sb = pp.tile([1, BT], f32)
            nc.gpsimd.memset(ones_sb, 1.0)
            h_fm = pp.tile([H, bc], f32)  # hidden state, feature-major
            c_fm = pp.tile([H, bc], f32)  # cell state, feature-major
            lpr_fm = pp.tile([2, bc], f32)
            nc.sync.dma_start(out=lpr_fm[:, :], in_=lpr_d.rearrange("b x -> x b"))

            # ---- init: transpose hh0/ch0 (bc,H) -> (H,bc) via PE ----
            with (
                tc.tile_pool(name="ps_init", bufs=4, space="PSUM") as psi,
                tc.tile_pool(name="sb_init", bufs=4) as sbi,
            ):
                for src_d, dst in ((hh_d, h_fm), (ch_d, c_fm)):
                    for j in range(bc // 128):
                        bm = sbi.tile([128, H], f32, tag="bm")
                        nc.sync.dma_start(out=bm[:, :], in_=src_d[j * 128 : (j + 1) * 128, :])
                        pt = psi.tile([128, 128], f32, tag="pt")
                        nc.tensor.transpose(pt[:, :], bm[:, :], ident[:, :])
                        nc.vector.tensor_copy(dst[:, j * 128 : (j + 1) * 128], pt[:, :])

            tc.strict_bb_all_engine_barrier()

            # ---- main scan ----
            with (
                tc.tile_pool(name="ps", bufs=2, space="PSUM") as ps,
                tc.tile_pool(name="sb", bufs=4) as sb,
            ):
                for t in range(t_steps):
                    for j in range(nbt):
                        bs = slice(j * BT, (j + 1) * BT)
                        hv = h_fm[:, bs]
                        cv = c_fm[:, bs]
                        g = ps.tile([128, 4 * BT], f32, tag="g")
                        for q in range(4):
                            o = g[:, q * BT : (q + 1) * BT]
                            ws = slice(q * 128, (q + 1) * 128)
                            if t == 0:
                                nc.tensor.matmul(o, whh_sb[:, ws], hv, start=True, stop=False)
                                nc.tensor.matmul(
                                    o, w0_sb[:, ws], lpr_fm[:, bs], start=False, stop=False
                                )
                            else:
                                nc.tensor.matmul(o, weff_sb[:, ws], hv, start=True, stop=False)
                            nc.tensor.matmul(
                                o, bias_sb[0:1, ws], ones_sb[0:1, :], start=False, stop=True
                            )
                        # sigmoid over [i|f|o], tanh over g  (one table set)
                        s3 = sb.tile([128, 3 * BT], f32, tag="s3")
                        nc.scalar.activation(s3[:, :], g[:, 0 : 3 * BT], AF.Sigmoid)
                        tg = sb.tile([128, BT], f32, tag="tg")
                        nc.scalar.activation(tg[:, :], g[:, 3 * BT : 4 * BT], AF.Tanh)
                        # cell update
                        tt = sb.tile([128, BT], f32, tag="tt")
                        nc.gpsimd.tensor_mul(tt[:, :], s3[:, 0:BT], tg[:, :])  # sig(i)*tanh(g)
                        u = sb.tile([128, BT], f32, tag="u")
                        nc.gpsimd.tensor_mul(u[:, :], s3[:, BT : 2 * BT], cv)  # sig(f)*c
                        nc.vector.tensor_add(cv, tt[:, :], u[:, :])  # c' (in place)
                        tc2 = sb.tile([128, BT], f32, tag="tc2")
                        nc.scalar.activation(tc2[:, :], cv, AF.Tanh)
                        nc.vector.tensor_mul(hv, s3[:, 2 * BT : 3 * BT], tc2[:, :])  # h' in place
                        # rel = w_pos @ h' into the consumed i-gate PSUM region
                        nc.tensor.matmul(g[0:2, 0:BT], wpos_sb[:, :], hv, start=True, stop=True)
                        rstage = sb.tile([2, BT], f32, tag="rstage")
                        nc.vector.tensor_copy(rstage[:, :], g[0:2, 0:BT])
                        nc.sync.dma_start(
                            out=rels_d[t, bs, :].rearrange("b x -> x b"),
                            in_=rstage[:, :],
                        )

            tc.strict_bb_all_engine_barrier()

            # ---- final: h_fm -> batch-major -> hN ----
            with (
                tc.tile_pool(name="ps_fin", bufs=4, space="PSUM") as psf,
                tc.tile_pool(name="sb_fin", bufs=4) as sbf,
            ):
                for j in range(bc // 128):
                    pt = psf.tile([128, 128], f32, tag="ptf")
                    nc.tensor.transpose(pt[:, :], h_fm[:, j * 128 : (j + 1) * 128], ident[:, :])
                    st = sbf.tile([128, 128], f32, tag="stf")
                    nc.vector.tensor_copy(st[:, :], pt[:, :])
                    nc.sync.dma_start(out=hN_d[j * 128 : (j + 1) * 128, :], in_=st[:, :])

    nc.finalize()
    return nc


def _fold_weights(w_ih, w_hh, b_ih, b_hh, w_emb, b_emb, w_pos, b_pos):
    """Fold the spatial-embedding/hidden2pos linears into the LSTM weights.
    Gate reorder [i,f,g,o] -> [i,f,o,g] so sigmoid gates are contiguous."""
    W_eff = w_hh + w_ih @ (w_emb @ w_pos)  # (4H, H)
    b_eff = b_ih + b_hh + w_ih @ (b_emb + w_emb @ b_pos)  # (4H,)
    W0 = w_ih @ w_emb  # (4H, 2)
    idx = np.concatenate(
        [np.arange(0, H), np.arange(H, 2 * H), np.arange(3 * H, 4 * H), np.arange(2 * H, 3 * H)]
    )
    weffT = np.ascontiguousarray(W_eff[idx].T, dtype=np.float32)  # (H, 4H)
    whhT = np.ascontiguousarray(w_hh[idx].T, dtype=np.float32)
    w0T = np.ascontiguousarray(W0[idx].T, dtype=np.float32)  # (2, 4H)
    wposT = np.ascontiguousarray(w_pos.T, dtype=np.float32)  # (H, 2)
    biasT = np.ascontiguousarray(b_eff[idx][None, :], dtype=np.float32)  # (1, 4H)
    return weffT, whhT, w0T, wposT, biasT


def kernel(
    last_pos,
    last_pos_rel,
    hh,
    ch,
    seq_start_end,
    w_ih,
    w_hh,
    b_ih,
    b_hh,
    w_emb,
    b_emb,
    w_pos,
    b_pos,
):
    global LAST_RESULT, LAST_NC
    from concourse.bass_utils import run_bass_kernel_spmd

    f = lambda a: np.asarray(a, dtype=np.float32)
    hh0 = f(hh)[0]
    ch0 = f(ch)[0]
    lpr = f(last_pos_rel)
    folded = _fold_weights(
        f(w_ih), f(w_hh), f(b_ih), f(b_hh), f(w_emb), f(b_emb), f(w_pos), f(b_pos)
    )

    nc = _build_program(*folded)
    LAST_NC = nc

    in_maps = []
    for c in range(NCORES):
        sl = slice(c * BC, (c + 1) * BC)
        in_maps.append(
            {
                "hh0": np.ascontiguousarray(hh0[sl]),
                "ch0": np.ascontiguousarray(ch0[sl]),
                "lpr": np.ascontiguousarray(lpr[sl]),
            }
        )

    res = run_bass_kernel_spmd(nc, in_maps, core_ids=list(range(NCORES)))
    LAST_RESULT = res

    rels = np.concatenate([r["rels"] for r in res.results], axis=1)
    hN = np.concatenate([r["hN"] for r in res.results], axis=0)
    return rels, hN[None]


# revision 26
# speedup vs baseline: 1.2795x; 1.2795x over previous
"""Trainium2 Bass kernel for the social-GAN LSTM decoder.

Full-input contract: kernel(**inputs) takes the unsharded numpy inputs and
returns (pred_traj_fake_rel (T,B,2), h_final (1,B,H)) exactly like the
reference. Internally: pure data-parallel over the batch across 8 cores;
host does the shard/layout prep (transposes), device does all the math.

Math notes (exact algebra, fp32):
  reference per step:
    gates = x @ w_ih.T + h @ w_hh.T + (b_ih + b_hh)
    i,f,g,o = split(gates); c' = sig(f)*c + sig(i)*tanh(g); h' = sig(o)*tanh(c')
    rel = h' @ w_pos.T + b_pos
    x' = rel @ w_emb.T + b_emb
  The (last_pos -> cur) chain never feeds any output, so it is dropped.
  x_{t+1} is linear in h_t, so for t>=1:
    gates_t = h_t @ W_eff.T + b_eff
      W_eff = w_hh + w_ih @ (w_emb @ w_pos)
      b_eff = b_ih + b_hh + w_ih @ (b_emb + w_emb @ b_pos)
  and for t=0 (x0 from last_pos_rel):
    gates_0 = h0 @ w_hh.T + last_pos_rel @ W0.T + b_eff,  W0 = w_ih @ w_emb

Device layout: feature-major. h,c live in SBUF as (H=128 partitions, batch
free), one tile per 512-batch chunk. Gate order is [g | i f o]: the g-gate
(tanh, bias via the ACT bias port) gets a 1-bank PSUM tile; i,f,o live in a
3-bank PSUM tile with their biases pre-accumulated by K=1 float32r matmuls
so ONE fused Sigmoid op covers all 1536 columns. All matmul operands are
float32r views (1 cycle/row on the PE vs 4 for float32, same 4-byte data).
rel_t = w_pos @ h_t is an M=2 matmul in its own PSUM bank, staged to SBUF
in pairs and DMA'd out contiguously as (T, 2, bc); the host untransposes.
"""

import os
import sys

import numpy as np

for _p in ("/opt/trn_rl_repo", "/root/.axon_site/_ro/trn_rl_repo"):
    if os.path.isdir(_p) and _p not in sys.path:
        try:
            import concourse.bass  # noqa: F401

            break
        except Exception:
            sys.path.insert(0, _p)

B, T, EMB, H = 65536, 32, 64, 128
NCORES = 8
BC = B // NCORES  # batch per core
BT = 512  # batch tile (free dim per matmul / PSUM bank)
G = 4 * H

LAST_RESULT = None  # BassKernelResults of the most recent run (for test.py)
LAST_NC = None  # built Bass program of the most recent run (for benching)
LAST_IN_MAPS = None  # per-core input dicts of the most recent run (for benching)


def _build_program(weffT, whhT, w0T, wposT, biasMM, biasG, bc=BC, t_steps=T):
    """Build the per-core Bass/Tile program. Weights are baked into the NEFF
    as Const tensors; per-core tensors hh0T/ch0T/lprT are ExternalInputs
    (already transposed to feature-major by the host)."""
    import concourse.mybir as mybir
    from concourse import bacc
    from concourse.tile import TileContext

    f32 = mybir.dt.float32
    f32r = mybir.dt.float32r
    AF = mybir.ActivationFunctionType
    nbt = bc // BT

    nc = bacc.Bacc("TRN2", target_bir_lowering=False, debug=False)

    hh_d = nc.dram_tensor("hh0T", [H, bc], f32r, kind="ExternalInput")
    ch_d = nc.dram_tensor("ch0T", [H, bc], f32, kind="ExternalInput")
    lpr_d = nc.dram_tensor("lprT", [2, bc], f32r, kind="ExternalInput")
    rels_d = nc.dram_tensor("relsT", [t_steps, 2, bc], f32, kind="ExternalOutput")
    hN_d = nc.dram_tensor("hNT", [H, bc], f32r, kind="ExternalOutput")

    # Everything that feeds a matmul is float32r end-to-end (same bytes as
    # f32; dt.np(float32r) is np.float32 so the host binds plain f32 arrays).
    weff_dr = nc.dram_tensor("weffT", [H, G], f32r, kind="ExternalInput")
    whh_dr = nc.dram_tensor("whhT", [H, G], f32r, kind="ExternalInput")
    w0_dr = nc.dram_tensor("w0T", [2, G], f32r, kind="ExternalInput")
    wpos_dr = nc.dram_tensor("wposT", [H, 2], f32r, kind="ExternalInput")
    biasmm_dr = nc.dram_tensor("biasMM", [1, G], f32r, kind="ExternalInput")
    ones_dr = nc.dram_tensor("ones", [1, BT], f32r, kind="ExternalInput")
    biasg_dr = nc.inline_tensor(biasG, "biasG")  # (H, 1)  ACT bias for tanh(g), f32

    with TileContext(nc) as tc:
        with tc.tile_pool(name="persist", bufs=1) as pp:
            weff_sb = pp.tile_from(weff_dr[:, :])
            whh_sb = pp.tile_from(whh_dr[:, :])
            w0_sb = pp.tile_from(w0_dr[:, :])
            wpos_sb = pp.tile_from(wpos_dr[:, :])
            biasmm_sb = pp.tile_from(biasmm_dr[:, :])
            biasg_sb = pp.tile_from(biasg_dr[:, :])
            ones_sb = pp.tile_from(ones_dr[:, :])
            # hidden/cell state, feature-major, one tile per batch-tile
            h_t = [pp.tile([H, BT], f32r, name=f"h{j}") for j in range(nbt)]
            c_p = [pp.tile([H, 2 * BT], f32, name=f"c{j}") for j in range(nbt // 2)]
            for j in range(nbt):
                bs = slice(j * BT, (j + 1) * BT)
                nc.sync.dma_start(out=h_t[j][:, :], in_=hh_d[:, bs])
                nc.sync.dma_start(
                    out=c_p[j // 2][:, (j % 2) * BT : (j % 2) * BT + BT], in_=ch_d[:, bs]
                )

            # ---- main scan ----
            with (
                tc.tile_pool(name="ps", bufs=2, space="PSUM") as ps,
                tc.tile_pool(name="sb", bufs=4) as sb,
            ):

                def gate_mm(o, t, j, q, hvr, with_bias, lprv=None):
                    ws = slice(q * 128, (q + 1) * 128)
                    if t == 0:
                        nc.tensor.matmul(
                            o, whh_sb[:, ws], hvr, start=True, stop=False
                        )
                        nc.tensor.matmul(
                            o,
                            w0_sb[:, ws],
                            lprv,
                            start=False,
                            stop=not with_bias,
                        )
                    else:
                        nc.tensor.matmul(
                            o, weff_sb[:, ws], hvr, start=True, stop=not with_bias
                        )
                    if with_bias:
                        nc.tensor.matmul(
                            o,
                            biasmm_sb[0:1, ws],
                            ones_sb[0:1, :],
                            start=False,
                            stop=True,
                        )

                rstage = None
                for t in range(t_steps):
                    for j in range(nbt):
                        hv = h_t[j][:, :]
                        cv = c_p[j // 2][:, (j % 2) * BT : (j % 2) * BT + BT]
                        hvr = hv
                        lprv = None
                        if t == 0:
                            lp = sb.tile([2, BT], f32r, tag="lpr", bufs=2)
                            nc.sync.dma_start(
                                out=lp[:, :], in_=lpr_d[:, j * BT : (j + 1) * BT]
                            )
                            lprv = lp[:, :]
                        # g-gate: 1-bank tile, tanh bias via ACT bias port
                        gg = ps.tile([128, BT], f32, tag="gg", bufs=1)
                        gate_mm(gg[:, :], t, j, 0, hvr, with_bias=False, lprv=lprv)
                        tg = sb.tile([128, BT], f32, tag="tg", bufs=6)
                        nc.scalar.activation(tg[:, :], gg[:, :], AF.Tanh, bias=biasg_sb[:, 0:1])
                        # i,f,o: 3-bank tile, biases pre-added via K=1 matmuls,
                        # one fused sigmoid over all 1536 columns
                        g3 = ps.tile([128, 3 * BT], f32, tag="g3", bufs=2)
                        for q in range(3):
                            gate_mm(
                                g3[:, q * BT : (q + 1) * BT],
                                t,
                                j,
                                q + 1,
                                hvr,
                                with_bias=True,
                                lprv=lprv,
                            )
                        s3 = sb.tile([128, 3 * BT], f32, tag="s3", bufs=6)
                        nc.scalar.activation(s3[:, :], g3[:, :], AF.Sigmoid)
                        # cell update
                        tt = sb.tile([128, BT], f32, tag="tt", bufs=6)
                        nc.gpsimd.tensor_mul(tt[:, :], s3[:, 0:BT], tg[:, :])  # sig(i)*tanh(g)
                        u = sb.tile([128, BT], f32, tag="u", bufs=6)
                        nc.gpsimd.tensor_mul(u[:, :], s3[:, BT : 2 * BT], cv)  # sig(f)*c
                        nc.vector.tensor_add(cv, tt[:, :], u[:, :])  # c' (in place)
                        if j % 2 == 1:
                            # one tanh over the c pair (both halves just updated)
                            tc2 = sb.tile([128, 2 * BT], f32, tag="tc2", bufs=4)
                            nc.scalar.activation(tc2[:, :], c_p[j // 2][:, :], AF.Tanh)
                            nc.vector.tensor_mul(
                                h_t[j - 1][:, :], s3_prev[:, 2 * BT : 3 * BT], tc2[:, 0:BT]
                            )  # h'(j-1)
                            nc.vector.tensor_mul(hv, s3[:, 2 * BT : 3 * BT], tc2[:, BT:])  # h'(j)
                        s3_prev = s3
                        if j % 2 == 1:
                            # rel for both tiles of the pair -> PSUM -> stage -> DRAM
                            rstage = sb.tile([2, 2 * BT], f32, tag="rstage")
                            for jj in (j - 1, j):
                                rel = ps.tile([2, BT], f32, tag="rel", bufs=1)
                                nc.tensor.matmul(
                                    rel[:, :], wpos_sb[:, :], h_t[jj][:, :], start=True, stop=True
                                )
                                half = slice((jj % 2) * BT, (jj % 2) * BT + BT)
                                nc.vector.tensor_copy(rstage[:, half], rel[:, :])
                            nc.sync.dma_start(
                                out=rels_d[t, :, (j - 1) * BT : (j + 1) * BT],
                                in_=rstage[:, :],
                            )

            # ---- final: h out in feature-major; host untransposes ----
            for j in range(nbt):
                nc.sync.dma_start(out=hN_d[:, j * BT : (j + 1) * BT], in_=h_t[j][:, :])

    nc.finalize()
    return nc


def _fold_weights(w_ih, w_hh, b_ih, b_hh, w_emb, b_emb, w_pos, b_pos):
    """Fold the spatial-embedding/hidden2pos linears into the LSTM weights.
    Gate reorder [i,f,g,o] -> [g,i,f,o]."""
    W_eff = w_hh + w_ih @ (w_emb @ w_pos)  # (4H, H)
    b_eff = b_ih + b_hh + w_ih @ (b_emb + w_emb @ b_pos)  # (4H,)
    W0 = w_ih @ w_emb  # (4H, 2)
    idx = np.concatenate(
        [np.arange(2 * H, 3 * H), np.arange(0, H), np.arange(H, 2 * H), np.arange(3 * H, 4 * H)]
    )
    weffT = np.ascontiguousarray(W_eff[idx].T, dtype=np.float32)  # (H, 4H)
    whhT = np.ascontiguousarray(w_hh[idx].T, dtype=np.float32)
    w0T = np.ascontiguousarray(W0[idx].T, dtype=np.float32)  # (2, 4H)
    wposT = np.ascontiguousarray(w_pos.T, dtype=np.float32)  # (H, 2)
    be = b_eff[idx].astype(np.float32)
    biasMM = np.ascontiguousarray(be[None, :])  # (1, 4H); [0:H] unused (g via ACT)
    biasG = np.ascontiguousarray(be[0:H, None])  # (H, 1) tanh-gate bias
    return weffT, whhT, w0T, wposT, biasMM, biasG


def kernel(
    last_pos,
    last_pos_rel,
    hh,
    ch,
    seq_start_end,
    w_ih,
    w_hh,
    b_ih,
    b_hh,
    w_emb,
    b_emb,
    w_pos,
    b_pos,
):
    global LAST_RESULT, LAST_NC, LAST_IN_MAPS
    from concourse.bass_utils import run_bass_kernel_spmd

    f = lambda a: np.asarray(a, dtype=np.float32)
    hh0T = np.ascontiguousarray(f(hh)[0].T)  # (H, B)
    ch0T = np.ascontiguousarray(f(ch)[0].T)
    lprT = np.ascontiguousarray(f(last_pos_rel).T)  # (2, B)
    folded = _fold_weights(
        f(w_ih), f(w_hh), f(b_ih), f(b_hh), f(w_emb), f(b_emb), f(w_pos), f(b_pos)
    )

    nc = _build_program(*folded)
    LAST_NC = nc

    weffT, whhT, w0T, wposT, biasMM, biasG = folded
    ones = np.ones((1, BT), np.float32)
    in_maps = []
    for c in range(NCORES):
        sl = slice(c * BC, (c + 1) * BC)
        in_maps.append(
            {
                "hh0T": np.ascontiguousarray(hh0T[:, sl]),
                "ch0T": np.ascontiguousarray(ch0T[:, sl]),
                "lprT": np.ascontiguousarray(lprT[:, sl]),
                "weffT": weffT,
                "whhT": whhT,
                "w0T": w0T,
                "wposT": wposT,
                "biasMM": biasMM,
                "ones": ones,
            }
        )

    LAST_IN_MAPS = in_maps
    res = run_bass_kernel_spmd(nc, in_maps, core_ids=list(range(NCORES)))
    LAST_RESULT = res

    # unshard + untranspose
    rels = np.concatenate(
        [r["relsT"].transpose(0, 2, 1) for r in res.results], axis=1
    )  # (T, B, 2)
    hN = np.concatenate([r["hNT"].T for r in res.results], axis=0)  # (B, H)
    return rels, hN[None]
